# revision 22
# baseline (speedup 1.0000x reference)
"""Trainium2 Bass kernel for nn_Attention3D: RMSNorm3D + 8-head attention + out-proj.

Sharding: 16 (b, h) slices over 8 cores -> each core gets one batch b and two
heads (h0, h0+1). Per-core weights are sliced/folded on the host; the device
does norm + qkv + attention + its partial output projection; host sums the 4
partial y's per batch.
"""
from contextlib import ExitStack

import numpy as np

import concourse.bass as bass
import concourse.tile as tile
from concourse import bacc, mybir
from concourse.bass_utils import run_bass_kernel_spmd

F32 = mybir.dt.float32
F32R = mybir.dt.float32r
AF = mybir.ActivationFunctionType


B, C, H, W, D = 2, 256, 16, 16, 16
N = H * W * D            # 4096
HEADS, DH = 8, 64
HID = HEADS * DH         # 512
NCORES = 8

ICH = 1024               # query-chunk (free dim of scores psum)
NIC = N // ICH           # 4
NJ = N // 128            # 32 key tiles
NT = N // 128            # 32 n-tiles (for col-form norm)


def build_attention_body(nc, tc, ctx, xb, wa, wv, wo, y):
    """Emits the whole per-core program.

    Setup (norm + qkv projections + V staging) is chunked into 8 per-512-
    column "bundles"; bundles 0-1 run as a prefix and the rest are emitted
    just-in-time inside the first attention stage's key loop so the PE and
    DVE work hides under the ScalarE exp stream (the kernel bottleneck).
    """
    const = ctx.enter_context(tc.tile_pool(name="const", bufs=1))
    work = ctx.enter_context(tc.tile_pool(name="work", bufs=2))
    pwork = ctx.enter_context(tc.tile_pool(name="pwork", bufs=3))
    psum = ctx.enter_context(tc.tile_pool(name="psum", bufs=2, space="PSUM"))

    # ---- constants / big slabs ----
    x_sb = const.tile([128, 2, N], F32R, tag="x")          # x, c-tile major
    wa_sb = const.tile([128, 2, 256], F32R, tag="wa")      # W_A^T  [c, (ct, o)]
    wv_sb = const.tile([128, 2, 256], F32R, tag="wv")      # W_V^T zero-padded to 256
    wo_sb = const.tile([64, 2, 256], F32R, tag="wo")       # W_O^T per head [d, h, c]
    ones_col = const.tile([128, 1], F32R, tag="onesc")
    ones_row = const.tile([1, 128], F32R, tag="onesr")
    invn_row = const.tile([1, N], F32R, tag="invr")        # 1/||x_n|| as a row
    invn_col = const.tile([128, NT], F32, tag="invc")      # same, tiled [p, t]
    inv_bcast = const.tile([128, N], F32, tag="invb")      # invn replicated over partitions
    qk_slab = const.tile([128, 2, N], F32R, tag="qk")      # [:,0,:]=q^T  [:,1,:]=k^T
    v_slab = const.tile([128, NJ, 2, 65], F32R, tag="v")   # per j-tile: [v_h | ones] x 2

    ONE_F32_BITS = 0x3F800000
    U32 = mybir.dt.uint32
    nc.vector.memset(ones_col[:].bitcast(U32), ONE_F32_BITS)
    nc.vector.memset(ones_row[:].bitcast(U32), ONE_F32_BITS)
    nc.vector.memset(v_slab[:, :, :, 64:65].bitcast(U32), ONE_F32_BITS)

    # ---- input DMAs (x chunk-ordered to feed the setup bundles; weights
    # interleave after the first chunks since they are needed a bit later) ----
    for ch in range(2):
        for ct in range(2):
            nc.sync.dma_start(
                out=x_sb[:, ct, ch * 512:(ch + 1) * 512],
                in_=xb[ct * 128:(ct + 1) * 128, ch * 512:(ch + 1) * 512],
            )
    for ct in range(2):
        nc.sync.dma_start(out=wa_sb[:, ct, :], in_=wa[ct * 128:(ct + 1) * 128, :])
        nc.sync.dma_start(out=wv_sb[:, ct, :], in_=wv[ct * 128:(ct + 1) * 128, :])
    nc.sync.dma_start(out=wo_sb[:, :, :], in_=wo.rearrange("(d h) c -> d h c", h=2))
    for ch in range(2, 8):
        for ct in range(2):
            nc.sync.dma_start(
                out=x_sb[:, ct, ch * 512:(ch + 1) * 512],
                in_=xb[ct * 128:(ct + 1) * 128, ch * 512:(ch + 1) * 512],
            )

    def norm_bundle(ch, ptag):
        """x2 + both norm orientations + invn broadcast for one 512-col chunk.
        Contains the only ACT (Sqrt) ops - keep these in the prefix so the
        exp table is loaded exactly once for the attention stream."""
        sl = bass.ts(ch, 512)
        x2c = [work.tile([128, 512], F32R, tag="x2", name=f"x2_{ch}_{i}")
               for i in range(2)]
        for ct in range(2):
            nc.scalar.activation(out=x2c[ct][:], in_=x_sb[:, ct, sl], func=AF.Square)
        nr_ps = psum.tile([1, 512], F32, tag=ptag, name=f"nr_ps_{ch}")
        for ct in range(2):
            nc.tensor.matmul(nr_ps[:], ones_col[:], x2c[ct][:],
                             start=(ct == 0), stop=(ct == 1))
        nrm_c = work.tile([1, 512], F32, tag="nr", name=f"nrm_c_{ch}")
        nc.scalar.activation(out=nrm_c[:], in_=nr_ps[:], func=AF.Sqrt)
        nc.vector.reciprocal(out=invn_row[0:1, sl], in_=nrm_c[:])
        ncol_ps = psum.tile([128, 4], F32, tag=ptag, name=f"ncol_ps_{ch}")
        for tt in range(4):
            for ct in range(2):
                nc.tensor.matmul(ncol_ps[:, tt:tt + 1],
                                 x2c[ct][:, tt * 128:(tt + 1) * 128].bitcast(F32),
                                 ones_col[:].bitcast(F32),
                                 start=(ct == 0), stop=(ct == 1))
        ncol_sb = work.tile([128, 4], F32, tag="ncs", name=f"ncol_sb_{ch}")
        nc.scalar.activation(out=ncol_sb[:], in_=ncol_ps[:], func=AF.Sqrt)
        nc.vector.reciprocal(out=invn_col[:, ch * 4:(ch + 1) * 4], in_=ncol_sb[:])
        ib_ps = psum.tile([128, 512], F32, tag=ptag, name=f"ib_ps_{ch}")
        nc.tensor.matmul(ib_ps[:], ones_row[:], invn_row[0:1, sl])
        nc.vector.tensor_copy(inv_bcast[:, sl], ib_ps[:])

    def proj_bundle(ch, ptag):
        """q/k/V projections + staging for one 512-col chunk (PE/DVE only)."""
        def emit():
            sl = bass.ts(ch, 512)
            for mt in range(2):
                qk_ps = psum.tile([128, 512], F32, tag=ptag, name=f"qk_ps_{ch}_{mt}")
                for ct in range(2):
                    nc.tensor.matmul(qk_ps[:], wa_sb[:, ct, mt * 128:(mt + 1) * 128],
                                     x_sb[:, ct, sl], start=(ct == 0), stop=(ct == 1))
                nc.vector.tensor_mul(qk_slab[:, mt, sl], qk_ps[:], inv_bcast[:, sl])
            for tt in range(4):
                t = ch * 4 + tt
                v_ps = psum.tile([128, 256], F32, tag=ptag, name=f"v_ps_{t}")
                for ct in range(2):
                    nc.tensor.matmul(v_ps[:], x_sb[:, ct, t * 128:(t + 1) * 128],
                                     wv_sb[:, ct, :], start=(ct == 0), stop=(ct == 1))
                for h in range(2):
                    nc.vector.tensor_scalar_mul(out=v_slab[:, t, h, 0:64],
                                                in0=v_ps[:, h * 64:(h + 1) * 64],
                                                scalar1=invn_col[:, t:t + 1])
        return emit

    def finalize_stage(ic, h, o_ps, o_slab):
        def emit():
            recd = work.tile([1, ICH], F32R, tag="rd", name=f"rd_{ic}_{h}")
            nc.vector.reciprocal(out=recd[:], in_=o_ps[64:65, :])
            bc_ps = psum.tile([64, ICH], F32, tag="s", name=f"bc_ps_{ic}_{h}")
            for hf in range(2):
                nc.tensor.matmul(bc_ps[:, hf * 512:(hf + 1) * 512],
                                 ones_row[0:1, 0:64],
                                 recd[0:1, hf * 512:(hf + 1) * 512])
            bc_sb = work.tile([64, ICH], F32, tag="bc", name=f"bc_sb_{ic}_{h}")
            nc.vector.tensor_copy(bc_sb[:], bc_ps[:])
            nc.vector.tensor_mul(o_slab[:, h, :], o_ps[0:64, :], bc_sb[:])
        return emit

    def outproj_stage(ic, o_slab):
        def emit():
            for mt in range(2):
                for cc in range(2):
                    y_ps = psum.tile([128, 512], F32, tag="o",
                                     name=f"y_ps_{ic}_{mt}_{cc}")
                    for h in range(2):
                        nc.tensor.matmul(y_ps[:],
                                         wo_sb[:, h, mt * 128:(mt + 1) * 128],
                                         o_slab[:, h, cc * 512:(cc + 1) * 512],
                                         start=(h == 0), stop=(h == 1))
                    y_ev = pwork.tile([128, 512], F32, tag="yev",
                                      name=f"y_ev_{ic}_{mt}_{cc}")
                    nc.vector.tensor_copy(y_ev[:], y_ps[:])
                    nc.sync.dma_start(
                        out=y[mt * 128:(mt + 1) * 128,
                              ic * ICH + cc * 512:ic * ICH + (cc + 1) * 512],
                        in_=y_ev[:])
        return emit

    nic = int(os.environ.get("K_NIC", NIC))
    stages = [(ic, h) for ic in range(nic) for h in range(2)]

    # prefix: all norm bundles (the only ACT-Sqrt users) + first two proj bundles
    for ch in range(8):
        norm_bundle(ch, "s" if ch % 2 else "o")
    proj_bundle(0, "s")()
    proj_bundle(1, "o")()

    # deferred emissions keyed by (stage_index, j)
    deferred = {}
    for ch in range(2, 8):
        # K chunk ch is first read by scores at j = 4*ch of stage 0
        deferred.setdefault((0, 4 * ch - 6), []).append(proj_bundle(ch, "o"))

    o_slab = None
    for si, (ic, h) in enumerate(stages):
        if h == 0:
            o_slab = work.tile([64, 2, ICH], F32R, tag="osl", name=f"osl_{ic}")
        hsl = slice(h * 64, (h + 1) * 64)
        o_ps = psum.tile([65, ICH], F32, tag="o", name=f"o_ps_{ic}_{h}")
        pq = {}
        for j in range(NJ):
            s_ps = psum.tile([128, ICH], F32, tag="s", name=f"s_{ic}_{h}_{j}")
            for hf in range(2):
                nc.tensor.matmul(
                    s_ps[:, hf * 512:(hf + 1) * 512],
                    qk_slab[hsl, 1, j * 128:(j + 1) * 128],
                    qk_slab[hsl, 0, ic * ICH + hf * 512:ic * ICH + (hf + 1) * 512],
                )
            for fn in deferred.pop((si, j), []):
                fn()
            if j >= 3:
                for hf in range(2):
                    nc.tensor.matmul(o_ps[:, hf * 512:(hf + 1) * 512],
                                     v_slab[:, j - 3, h, :],
                                     pq[j - 3][:, hf * 512:(hf + 1) * 512],
                                     start=(j - 3 == 0), stop=False)
            p_t = pwork.tile([128, ICH], F32R, tag="p", bufs=4,
                             name=f"p_{ic}_{h}_{j}")
            nc.scalar.activation(out=p_t[:], in_=s_ps[:], func=AF.Exp)
            pq[j] = p_t
        for jj in (NJ - 3, NJ - 2, NJ - 1):
            for hf in range(2):
                nc.tensor.matmul(o_ps[:, hf * 512:(hf + 1) * 512],
                                 v_slab[:, jj, h, :],
                                 pq[jj][:, hf * 512:(hf + 1) * 512],
                                 start=False, stop=(jj == NJ - 1))
        deferred.setdefault((si + 1, 2), []).append(
            finalize_stage(ic, h, o_ps, o_slab))
        if h == 1:
            deferred.setdefault((si + 1, 6), []).append(outproj_stage(ic, o_slab))
    for key in sorted(deferred):
        for fn in deferred[key]:
            fn()


_NC_CACHE = None


def _build():
    global _NC_CACHE
    if _NC_CACHE is not None:
        return _NC_CACHE
    nc = bacc.Bacc("TRN2", target_bir_lowering=False, debug=False, num_devices=NCORES)
    xb = nc.dram_tensor("xb", [C, N], F32R, kind="ExternalInput").ap()
    wa = nc.dram_tensor("wa", [C, 256], F32R, kind="ExternalInput").ap()
    wv = nc.dram_tensor("wv", [C, 256], F32R, kind="ExternalInput").ap()
    wo = nc.dram_tensor("wo", [128, C], F32R, kind="ExternalInput").ap()
    y = nc.dram_tensor("y", [C, N], F32, kind="ExternalOutput").ap()
    with tile.TileContext(nc) as tc, ExitStack() as ctx:
        with nc.allow_low_precision(reason="float32r rounding is within tolerance"):
            build_attention_body(nc, tc, ctx, xb, wa, wv, wo, y)
    nc.compile()
    _NC_CACHE = nc
    return nc


def _host_prep(x, g, w_qkv, w_out):
    """Per-core input maps."""
    x = np.ascontiguousarray(np.asarray(x, np.float32))
    g = np.asarray(g, np.float32)
    w_qkv = np.asarray(w_qkv, np.float32)
    w_out = np.asarray(w_out, np.float32)

    Wg = w_qkv * (g * np.sqrt(np.float32(C)))[None, :]
    Wq = Wg[0:HID] * np.float32(DH ** -0.5)
    Wk = Wg[HID:2 * HID]
    Wv = Wg[2 * HID:3 * HID]

    in_maps = []
    for core in range(NCORES):
        b = core // 4
        h0 = 2 * (core % 4)
        sl = slice(h0 * DH, (h0 + 2) * DH)
        W_A = np.concatenate([Wq[sl], Wk[sl]], 0)            # [256, 256]
        wo_slice = w_out[:, sl]                              # [256, 128]
        wo_dev = np.ascontiguousarray(
            wo_slice.T.reshape(2, DH, C).transpose(1, 0, 2).reshape(128, C))
        in_maps.append({
            "xb": np.ascontiguousarray(x[b].reshape(C, N)),
            "wa": np.ascontiguousarray(W_A.T),               # [c, o]
            "wv": np.ascontiguousarray(np.pad(Wv[sl].T, ((0, 0), (0, 128)))),  # [c, 256] zero-padded
            "wo": wo_dev,                                    # [(d,h), c]
        })
    return in_maps


_RUNNER_CACHE = None


def _make_runner(nc):
    """Build the sharded PJRT callable once; reuse across kernel() calls
    (run_bass_kernel_spmd re-traces jax on every invocation)."""
    import jax
    from jax.sharding import Mesh, PartitionSpec
    from jax.experimental.shard_map import shard_map
    from concourse import bass2jax

    bass2jax.install_neuronx_cc_hook()
    in_names, out_names, out_avals, zero_outs = [], [], [], []
    for alloc in nc.m.functions[0].allocations:
        if not isinstance(alloc, mybir.MemoryLocationSet):
            continue
        name = alloc.memorylocations[0].name
        if alloc.kind == "ExternalInput":
            if nc.partition_id_tensor is None or name != nc.partition_id_tensor.name:
                in_names.append(name)
        elif alloc.kind == "ExternalOutput":
            out_names.append(name)
            shape = tuple(alloc.tensor_shape)
            dtype = mybir.dt.np(alloc.dtype)
            out_avals.append(jax.core.ShapedArray(shape, dtype))
            zero_outs.append(np.zeros(shape, dtype))
    n_params = len(in_names)
    all_in_names = list(in_names) + list(out_names)
    if nc.partition_id_tensor is not None:
        all_in_names.append(nc.partition_id_tensor.name)

    def _body(*args):
        operands = list(args)
        if nc.partition_id_tensor is not None:
            operands.append(bass2jax.partition_id_tensor())
        return tuple(bass2jax._bass_exec_p.bind(
            *operands,
            out_avals=tuple(out_avals),
            in_names=tuple(all_in_names),
            out_names=tuple(out_names),
            lowering_input_output_aliases=(),
            sim_require_finite=True,
            sim_require_nnan=True,
            nc=nc,
        ))

    devices = jax.devices()[:NCORES]
    mesh = Mesh(np.asarray(devices), ("core",))
    n_outs = len(out_avals)
    fn = jax.jit(
        shard_map(_body, mesh=mesh,
                  in_specs=(PartitionSpec("core"),) * (n_params + n_outs),
                  out_specs=(PartitionSpec("core"),) * n_outs,
                  check_rep=False),
        keep_unused=True,
    )
    sharding = jax.sharding.NamedSharding(mesh, PartitionSpec("core"))
    dev_zero = [jax.device_put(
        np.zeros((NCORES * z.shape[0], *z.shape[1:]), z.dtype), sharding)
        for z in zero_outs]

    def run(in_maps):
        concat_in = [np.concatenate([np.asarray(m[name]) for m in in_maps], axis=0)
                     for name in in_names]
        dev_in = [jax.device_put(a, sharding) for a in concat_in]
        outs = fn(*dev_in, *dev_zero)
        y = np.asarray(outs[0]).reshape(NCORES, C, N)
        return y

    return run


def kernel(x, g, w_qkv, w_out, b_out):
    global _RUNNER_CACHE
    nc = _build()
    in_maps = _host_prep(x, g, w_qkv, w_out)
    try:
        if _RUNNER_CACHE is None:
            _RUNNER_CACHE = _make_runner(nc)
        y_cores = _RUNNER_CACHE(in_maps)
    except Exception:
        # fallback: the stock (re-tracing) path
        res = run_bass_kernel_spmd(nc, in_maps, core_ids=list(range(NCORES)))
        y_cores = np.stack([res.results[c]["y"] for c in range(NCORES)])
    y = np.zeros((B, C, N), np.float32)
    for core in range(NCORES):
        y[core // 4] += y_cores[core]
    y += np.asarray(b_out, np.float32)[None, :, None]
    return y.reshape(B, C, H, W, D)


# revision 30
# speedup vs baseline: 1.0078x; 1.0078x over previous
"""Trainium2 Bass kernel for nn_Attention3D: RMSNorm3D + 8-head attention + out-proj.

Sharding: 16 (b, h) slices over 8 cores -> each core gets one batch b and two
heads (h0, h0+1). Per-core weights are sliced/folded on the host; the device
does norm + qkv + attention + its partial output projection; host sums the 4
partial y's per batch.
"""
from contextlib import ExitStack

import numpy as np

import concourse.bass as bass
import concourse.tile as tile
from concourse import bacc, mybir
from concourse.bass_utils import run_bass_kernel_spmd

F32 = mybir.dt.float32
F32R = mybir.dt.float32r
AF = mybir.ActivationFunctionType


B, C, H, W, D = 2, 256, 16, 16, 16
N = H * W * D            # 4096
HEADS, DH = 8, 64
HID = HEADS * DH         # 512
NCORES = 8

ICH = 1024               # query-chunk (free dim of scores psum)
K_LEAD = 10              # how many j-iterations before first use to emit K chunks
V_LEAD = 4               # same for V tiles
Q_AT = 8                 # j-index where q bundles flush in their stage
NIC = N // ICH           # 4
NJ = N // 128            # 32 key tiles
NT = N // 128            # 32 n-tiles (for col-form norm)


def build_attention_body(nc, tc, ctx, xb, wa, wv, wo, y):
    """Emits the whole per-core program.

    Setup (norm + qkv projections + V staging) is chunked into 8 per-512-
    column "bundles"; bundles 0-1 run as a prefix and the rest are emitted
    just-in-time inside the first attention stage's key loop so the PE and
    DVE work hides under the ScalarE exp stream (the kernel bottleneck).
    """
    const = ctx.enter_context(tc.tile_pool(name="const", bufs=1))
    work = ctx.enter_context(tc.tile_pool(name="work", bufs=2))
    pwork = ctx.enter_context(tc.tile_pool(name="pwork", bufs=3))
    psum = ctx.enter_context(tc.tile_pool(name="psum", bufs=2, space="PSUM"))

    # ---- constants / big slabs ----
    x_sb = const.tile([128, 2, N], F32R, tag="x")          # x, c-tile major
    wa_sb = const.tile([128, 2, 256], F32R, tag="wa")      # W_A^T  [c, (ct, o)]
    wv_sb = const.tile([128, 2, 256], F32R, tag="wv")      # W_V^T zero-padded to 256
    wo_sb = const.tile([64, 2, 256], F32R, tag="wo")       # W_O^T per head [d, h, c]
    ones_col = const.tile([128, 1], F32R, tag="onesc")
    ones_row = const.tile([1, 128], F32R, tag="onesr")
    invn_row = const.tile([1, N], F32R, tag="invr")        # 1/||x_n|| as a row
    invn_col = const.tile([128, NT], F32, tag="invc")      # same, tiled [p, t]
    inv_bcast = const.tile([128, N], F32, tag="invb")      # invn replicated over partitions
    qk_slab = const.tile([128, 2, N], F32R, tag="qk")      # [:,0,:]=q^T  [:,1,:]=k^T
    v_slab = const.tile([128, NJ, 2, 65], F32R, tag="v")   # per j-tile: [v_h | ones] x 2

    ONE_F32_BITS = 0x3F800000
    U32 = mybir.dt.uint32
    nc.vector.memset(ones_col[:].bitcast(U32), ONE_F32_BITS)
    nc.vector.memset(ones_row[:].bitcast(U32), ONE_F32_BITS)
    nc.vector.memset(v_slab[:, :, :, 64:65].bitcast(U32), ONE_F32_BITS)

    # ---- input DMAs (x chunk-ordered to feed the setup bundles; weights
    # interleave after the first chunks since they are needed a bit later) ----
    _dma_eng = [nc.sync, nc.gpsimd]
    for ch in range(2):
        for ct in range(2):
            _dma_eng[ct].dma_start(
                out=x_sb[:, ct, ch * 512:(ch + 1) * 512],
                in_=xb[ct * 128:(ct + 1) * 128, ch * 512:(ch + 1) * 512],
            )
    for ct in range(2):
        nc.sync.dma_start(out=wa_sb[:, ct, :], in_=wa[ct * 128:(ct + 1) * 128, :])
        nc.gpsimd.dma_start(out=wv_sb[:, ct, :], in_=wv[ct * 128:(ct + 1) * 128, :])
    nc.sync.dma_start(out=wo_sb[:, :, :], in_=wo.rearrange("(d h) c -> d h c", h=2))
    for ch in range(2, 8):
        for ct in range(2):
            _dma_eng[ct].dma_start(
                out=x_sb[:, ct, ch * 512:(ch + 1) * 512],
                in_=xb[ct * 128:(ct + 1) * 128, ch * 512:(ch + 1) * 512],
            )

    def norm_bundle(ch, ptag):
        """x2 + both norm orientations + invn broadcast for one 512-col chunk.
        Contains the only ACT (Sqrt) ops - keep these in the prefix so the
        exp table is loaded exactly once for the attention stream."""
        sl = bass.ts(ch, 512)
        x2c = [work.tile([128, 512], F32R, tag="x2", name=f"x2_{ch}_{i}")
               for i in range(2)]
        for ct in range(2):
            nc.scalar.activation(out=x2c[ct][:], in_=x_sb[:, ct, sl], func=AF.Square)
        nr_ps = psum.tile([1, 512], F32, tag=ptag, name=f"nr_ps_{ch}")
        for ct in range(2):
            nc.tensor.matmul(nr_ps[:], ones_col[:], x2c[ct][:],
                             start=(ct == 0), stop=(ct == 1))
        nrm_c = work.tile([1, 512], F32, tag="nr", name=f"nrm_c_{ch}")
        nc.scalar.activation(out=nrm_c[:], in_=nr_ps[:], func=AF.Sqrt)
        nc.vector.reciprocal(out=invn_row[0:1, sl], in_=nrm_c[:])
        ncol_ps = psum.tile([128, 4], F32, tag=ptag, name=f"ncol_ps_{ch}")
        for tt in range(4):
            for ct in range(2):
                nc.tensor.matmul(ncol_ps[:, tt:tt + 1],
                                 x2c[ct][:, tt * 128:(tt + 1) * 128].bitcast(F32),
                                 ones_col[:].bitcast(F32),
                                 start=(ct == 0), stop=(ct == 1))
        ncol_sb = work.tile([128, 4], F32, tag="ncs", name=f"ncol_sb_{ch}")
        nc.scalar.activation(out=ncol_sb[:], in_=ncol_ps[:], func=AF.Sqrt)
        nc.vector.reciprocal(out=invn_col[:, ch * 4:(ch + 1) * 4], in_=ncol_sb[:])
        ib_ps = psum.tile([128, 512], F32, tag=ptag, name=f"ib_ps_{ch}")
        nc.tensor.matmul(ib_ps[:], ones_row[:], invn_row[0:1, sl])
        nc.vector.tensor_copy(inv_bcast[:, sl], ib_ps[:])

    def proj_bundle(ch, ptag):
        """q/k/V projections + staging for one 512-col chunk (PE/DVE only)."""
        def emit():
            sl = bass.ts(ch, 512)
            for mt in range(2):
                qk_ps = psum.tile([128, 512], F32, tag=ptag, name=f"qk_ps_{ch}_{mt}")
                for ct in range(2):
                    nc.tensor.matmul(qk_ps[:], wa_sb[:, ct, mt * 128:(mt + 1) * 128],
                                     x_sb[:, ct, sl], start=(ct == 0), stop=(ct == 1))
                nc.vector.tensor_mul(qk_slab[:, mt, sl], qk_ps[:], inv_bcast[:, sl])
            for tt in range(4):
                t = ch * 4 + tt
                v_ps = psum.tile([128, 256], F32, tag=ptag, name=f"v_ps_{t}")
                for ct in range(2):
                    nc.tensor.matmul(v_ps[:], x_sb[:, ct, t * 128:(t + 1) * 128],
                                     wv_sb[:, ct, :], start=(ct == 0), stop=(ct == 1))
                for h in range(2):
                    nc.vector.tensor_scalar_mul(out=v_slab[:, t, h, 0:64],
                                                in0=v_ps[:, h * 64:(h + 1) * 64],
                                                scalar1=invn_col[:, t:t + 1])
        return emit

    def finalize_stage(ic, h, o_ps, o_slab):
        def emit():
            recd = work.tile([1, ICH], F32R, tag="rd", name=f"rd_{ic}_{h}")
            nc.vector.reciprocal(out=recd[:], in_=o_ps[64:65, :])
            bc_ps = psum.tile([64, ICH], F32, tag="s", name=f"bc_ps_{ic}_{h}")
            for hf in range(2):
                nc.tensor.matmul(bc_ps[:, hf * 512:(hf + 1) * 512],
                                 ones_row[0:1, 0:64],
                                 recd[0:1, hf * 512:(hf + 1) * 512])
            bc_sb = work.tile([64, ICH], F32, tag="bc", name=f"bc_sb_{ic}_{h}")
            nc.vector.tensor_copy(bc_sb[:], bc_ps[:])
            nc.vector.tensor_mul(o_slab[:, h, :], o_ps[0:64, :], bc_sb[:])
        return emit

    def outproj_stage(ic, o_slab):
        def emit():
            for mt in range(2):
                for cc in range(2):
                    y_ps = psum.tile([128, 512], F32, tag="o",
                                     name=f"y_ps_{ic}_{mt}_{cc}")
                    for h in range(2):
                        nc.tensor.matmul(y_ps[:],
                                         wo_sb[:, h, mt * 128:(mt + 1) * 128],
                                         o_slab[:, h, cc * 512:(cc + 1) * 512],
                                         start=(h == 0), stop=(h == 1))
                    y_ev = pwork.tile([128, 512], F32, tag="yev",
                                      name=f"y_ev_{ic}_{mt}_{cc}")
                    nc.vector.tensor_copy(y_ev[:], y_ps[:])
                    nc.sync.dma_start(
                        out=y[mt * 128:(mt + 1) * 128,
                              ic * ICH + cc * 512:ic * ICH + (cc + 1) * 512],
                        in_=y_ev[:])
        return emit

    nic = int(os.environ.get("K_NIC", NIC))
    stages = [(ic, h) for ic in range(nic) for h in range(2)]

    # prefix: all norm bundles (the only ACT-Sqrt users) + first two proj bundles
    for ch in range(8):
        norm_bundle(ch, "s" if ch % 2 else "o")
    proj_bundle(0, "s")()
    proj_bundle(1, "o")()

    # deferred emissions keyed by (stage_index, j)
    deferred = {}
    for ch in range(2, 8):
        # K chunk ch is first read by scores at j = 4*ch of stage 0
        deferred.setdefault((0, 4 * ch - 6), []).append(proj_bundle(ch, "o"))

    o_slab = None
    for si, (ic, h) in enumerate(stages):
        if h == 0:
            o_slab = work.tile([64, 2, ICH], F32R, tag="osl", name=f"osl_{ic}")
        hsl = slice(h * 64, (h + 1) * 64)
        o_ps = psum.tile([65, ICH], F32, tag="o", name=f"o_ps_{ic}_{h}")
        pq = {}
        for j in range(NJ):
            s_ps = psum.tile([128, ICH], F32, tag="s", name=f"s_{ic}_{h}_{j}")
            for hf in range(2):
                nc.tensor.matmul(
                    s_ps[:, hf * 512:(hf + 1) * 512],
                    qk_slab[hsl, 1, j * 128:(j + 1) * 128],
                    qk_slab[hsl, 0, ic * ICH + hf * 512:ic * ICH + (hf + 1) * 512],
                )
            for fn in deferred.pop((si, j), []):
                fn()
            if j >= 3:
                for hf in range(2):
                    nc.tensor.matmul(o_ps[:, hf * 512:(hf + 1) * 512],
                                     v_slab[:, j - 3, h, :],
                                     pq[j - 3][:, hf * 512:(hf + 1) * 512],
                                     start=(j - 3 == 0), stop=False)
            p_t = pwork.tile([128, ICH], F32R, tag="p", bufs=4,
                             name=f"p_{ic}_{h}_{j}")
            nc.scalar.activation(out=p_t[:], in_=s_ps[:], func=AF.Exp)
            pq[j] = p_t
        def tail_avs(h, o_ps, pq):
            def emit():
                for jj in (NJ - 3, NJ - 2, NJ - 1):
                    for hf in range(2):
                        nc.tensor.matmul(o_ps[:, hf * 512:(hf + 1) * 512],
                                         v_slab[:, jj, h, :],
                                         pq[jj][:, hf * 512:(hf + 1) * 512],
                                         start=False, stop=(jj == NJ - 1))
            return emit
        deferred.setdefault((si + 1, 0), []).append(tail_avs(h, o_ps, pq))
        deferred.setdefault((si + 1, 2), []).append(
            finalize_stage(ic, h, o_ps, o_slab))
        if h == 1:
            deferred.setdefault((si + 1, 6), []).append(outproj_stage(ic, o_slab))
    for key in sorted(deferred):
        for fn in deferred[key]:
            fn()


_NC_CACHE = None


def _build():
    global _NC_CACHE
    if _NC_CACHE is not None:
        return _NC_CACHE
    nc = bacc.Bacc("TRN2", target_bir_lowering=False, debug=False, num_devices=NCORES)
    xb = nc.dram_tensor("xb", [C, N], F32R, kind="ExternalInput").ap()
    wa = nc.dram_tensor("wa", [C, 256], F32R, kind="ExternalInput").ap()
    wv = nc.dram_tensor("wv", [C, 256], F32R, kind="ExternalInput").ap()
    wo = nc.dram_tensor("wo", [128, C], F32R, kind="ExternalInput").ap()
    y = nc.dram_tensor("y", [C, N], F32, kind="ExternalOutput").ap()
    with tile.TileContext(nc) as tc, ExitStack() as ctx:
        with nc.allow_low_precision(reason="float32r rounding is within tolerance"):
            build_attention_body(nc, tc, ctx, xb, wa, wv, wo, y)
    nc.compile()
    _NC_CACHE = nc
    return nc


def _host_prep(x, g, w_qkv, w_out):
    """Per-core input maps."""
    x = np.ascontiguousarray(np.asarray(x, np.float32))
    g = np.asarray(g, np.float32)
    w_qkv = np.asarray(w_qkv, np.float32)
    w_out = np.asarray(w_out, np.float32)

    Wg = w_qkv * (g * np.sqrt(np.float32(C)))[None, :]
    Wq = Wg[0:HID] * np.float32(DH ** -0.5)
    Wk = Wg[HID:2 * HID]
    Wv = Wg[2 * HID:3 * HID]

    in_maps = []
    for core in range(NCORES):
        b = core // 4
        h0 = 2 * (core % 4)
        sl = slice(h0 * DH, (h0 + 2) * DH)
        W_A = np.concatenate([Wq[sl], Wk[sl]], 0)            # [256, 256]
        wo_slice = w_out[:, sl]                              # [256, 128]
        wo_dev = np.ascontiguousarray(
            wo_slice.T.reshape(2, DH, C).transpose(1, 0, 2).reshape(128, C))
        in_maps.append({
            "xb": np.ascontiguousarray(x[b].reshape(C, N)),
            "wa": np.ascontiguousarray(W_A.T),               # [c, o]
            "wv": np.ascontiguousarray(np.pad(Wv[sl].T, ((0, 0), (0, 128)))),  # [c, 256] zero-padded
            "wo": wo_dev,                                    # [(d,h), c]
        })
    return in_maps


_RUNNER_CACHE = None


def _make_runner(nc):
    """Build the sharded PJRT callable once; reuse across kernel() calls
    (run_bass_kernel_spmd re-traces jax on every invocation)."""
    import jax
    from jax.sharding import Mesh, PartitionSpec
    from jax.experimental.shard_map import shard_map
    from concourse import bass2jax

    bass2jax.install_neuronx_cc_hook()
    in_names, out_names, out_avals, zero_outs = [], [], [], []
    for alloc in nc.m.functions[0].allocations:
        if not isinstance(alloc, mybir.MemoryLocationSet):
            continue
        name = alloc.memorylocations[0].name
        if alloc.kind == "ExternalInput":
            if nc.partition_id_tensor is None or name != nc.partition_id_tensor.name:
                in_names.append(name)
        elif alloc.kind == "ExternalOutput":
            out_names.append(name)
            shape = tuple(alloc.tensor_shape)
            dtype = mybir.dt.np(alloc.dtype)
            out_avals.append(jax.core.ShapedArray(shape, dtype))
            zero_outs.append(np.zeros(shape, dtype))
    n_params = len(in_names)
    all_in_names = list(in_names) + list(out_names)
    if nc.partition_id_tensor is not None:
        all_in_names.append(nc.partition_id_tensor.name)

    def _body(*args):
        operands = list(args)
        if nc.partition_id_tensor is not None:
            operands.append(bass2jax.partition_id_tensor())
        return tuple(bass2jax._bass_exec_p.bind(
            *operands,
            out_avals=tuple(out_avals),
            in_names=tuple(all_in_names),
            out_names=tuple(out_names),
            lowering_input_output_aliases=(),
            sim_require_finite=True,
            sim_require_nnan=True,
            nc=nc,
        ))

    devices = jax.devices()[:NCORES]
    mesh = Mesh(np.asarray(devices), ("core",))
    n_outs = len(out_avals)
    fn = jax.jit(
        shard_map(_body, mesh=mesh,
                  in_specs=(PartitionSpec("core"),) * (n_params + n_outs),
                  out_specs=(PartitionSpec("core"),) * n_outs,
                  check_rep=False),
        keep_unused=True,
    )
    sharding = jax.sharding.NamedSharding(mesh, PartitionSpec("core"))
    dev_zero = [jax.device_put(
        np.zeros((NCORES * z.shape[0], *z.shape[1:]), z.dtype), sharding)
        for z in zero_outs]

    def run(in_maps):
        concat_in = [np.concatenate([np.asarray(m[name]) for m in in_maps], axis=0)
                     for name in in_names]
        dev_in = [jax.device_put(a, sharding) for a in concat_in]
        outs = fn(*dev_in, *dev_zero)
        y = np.asarray(outs[0]).reshape(NCORES, C, N)
        return y

    return run


def kernel(x, g, w_qkv, w_out, b_out):
    global _RUNNER_CACHE
    nc = _build()
    in_maps = _host_prep(x, g, w_qkv, w_out)
    try:
        if _RUNNER_CACHE is None:
            _RUNNER_CACHE = _make_runner(nc)
        y_cores = _RUNNER_CACHE(in_maps)
    except Exception:
        # fallback: the stock (re-tracing) path
        res = run_bass_kernel_spmd(nc, in_maps, core_ids=list(range(NCORES)))
        y_cores = np.stack([res.results[c]["y"] for c in range(NCORES)])
    y = np.zeros((B, C, N), np.float32)
    for core in range(NCORES):
        y[core // 4] += y_cores[core]
    y += np.asarray(b_out, np.float32)[None, :, None]
    return y.reshape(B, C, H, W, D)


# revision 37
# speedup vs baseline: 1.0124x; 1.0046x over previous
"""Trainium2 Bass kernel for nn_Attention3D: RMSNorm3D + 8-head attention + out-proj.

Sharding: 16 (b, h) slices over 8 cores -> each core gets one batch b and two
heads (h0, h0+1). Per-core weights are sliced/folded on the host; the device
does norm + qkv + attention + its partial output projection; host sums the 4
partial y's per batch.
"""
from contextlib import ExitStack

import numpy as np
import ml_dtypes as _ml

import concourse.bass as bass
import concourse.tile as tile
from concourse import bacc, mybir
from concourse.bass_utils import run_bass_kernel_spmd

F32 = mybir.dt.float32
F32R = mybir.dt.float32r
BF16 = mybir.dt.bfloat16
AF = mybir.ActivationFunctionType


B, C, H, W, D = 2, 256, 16, 16, 16
N = H * W * D            # 4096
HEADS, DH = 8, 64
HID = HEADS * DH         # 512
NCORES = 8

ICH = 1024               # query-chunk (free dim of scores psum)
K_LEAD = 10              # how many j-iterations before first use to emit K chunks
V_LEAD = 4               # same for V tiles
Q_AT = 8                 # j-index where q bundles flush in their stage
NIC = N // ICH           # 4
NJ = N // 128            # 32 key tiles
NT = N // 128            # 32 n-tiles (for col-form norm)


def build_attention_body(nc, tc, ctx, xb, wa, wv, wo, y):
    """Emits the whole per-core program.

    Setup (norm + qkv projections + V staging) is chunked into 8 per-512-
    column "bundles"; bundles 0-1 run as a prefix and the rest are emitted
    just-in-time inside the first attention stage's key loop so the PE and
    DVE work hides under the ScalarE exp stream (the kernel bottleneck).
    """
    const = ctx.enter_context(tc.tile_pool(name="const", bufs=1))
    work = ctx.enter_context(tc.tile_pool(name="work", bufs=2))
    pwork = ctx.enter_context(tc.tile_pool(name="pwork", bufs=3))
    psum = ctx.enter_context(tc.tile_pool(name="psum", bufs=2, space="PSUM"))

    # ---- constants / big slabs ----
    x_sb = const.tile([128, 2, N], BF16, tag="x")          # x, c-tile major
    wa_sb = const.tile([128, 2, 256], BF16, tag="wa")      # W_A^T  [c, (ct, o)]
    wv_sb = const.tile([128, 2, 256], BF16, tag="wv")      # W_V^T zero-padded to 256
    wo_sb = const.tile([64, 2, 256], F32R, tag="wo")       # W_O^T per head [d, h, c]
    ones_col = const.tile([128, 1], F32R, tag="onesc")
    ones_row = const.tile([1, 128], F32R, tag="onesr")
    invn_row = const.tile([1, N], F32R, tag="invr")        # 1/||x_n|| as a row
    invn_col = const.tile([128, NT], F32, tag="invc")      # same, tiled [p, t]
    inv_bcast = const.tile([128, N], F32, tag="invb")      # invn replicated over partitions
    qk_slab = const.tile([128, 2, N], F32R, tag="qk")      # [:,0,:]=q^T  [:,1,:]=k^T
    v_slab = const.tile([128, NJ, 2, 65], F32R, tag="v")   # per j-tile: [v_h | ones] x 2

    ONE_F32_BITS = 0x3F800000
    U32 = mybir.dt.uint32
    nc.vector.memset(ones_col[:].bitcast(U32), ONE_F32_BITS)
    nc.vector.memset(ones_row[:].bitcast(U32), ONE_F32_BITS)
    nc.vector.memset(v_slab[:, :, :, 64:65].bitcast(U32), ONE_F32_BITS)

    # ---- input DMAs (x chunk-ordered to feed the setup bundles; weights
    # interleave after the first chunks since they are needed a bit later) ----
    _dma_eng = [nc.sync, nc.gpsimd]
    for ch in range(2):
        for ct in range(2):
            _dma_eng[ct].dma_start(
                out=x_sb[:, ct, ch * 512:(ch + 1) * 512],
                in_=xb[ct * 128:(ct + 1) * 128, ch * 512:(ch + 1) * 512],
            )
    for ct in range(2):
        nc.sync.dma_start(out=wa_sb[:, ct, :], in_=wa[ct * 128:(ct + 1) * 128, :])
        nc.gpsimd.dma_start(out=wv_sb[:, ct, :], in_=wv[ct * 128:(ct + 1) * 128, :])
    nc.sync.dma_start(out=wo_sb[:, :, :], in_=wo.rearrange("(d h) c -> d h c", h=2))
    for ch in range(2, 8):
        for ct in range(2):
            _dma_eng[ct].dma_start(
                out=x_sb[:, ct, ch * 512:(ch + 1) * 512],
                in_=xb[ct * 128:(ct + 1) * 128, ch * 512:(ch + 1) * 512],
            )

    def norm_bundle(ch, ptag):
        """x2 + both norm orientations + invn broadcast for one 512-col chunk.
        Contains the only ACT (Sqrt) ops - keep these in the prefix so the
        exp table is loaded exactly once for the attention stream."""
        sl = bass.ts(ch, 512)
        x2c = [work.tile([128, 512], F32R, tag="x2", name=f"x2_{ch}_{i}")
               for i in range(2)]
        for ct in range(2):
            nc.scalar.activation(out=x2c[ct][:], in_=x_sb[:, ct, sl], func=AF.Square)
        nr_ps = psum.tile([1, 512], F32, tag=ptag, name=f"nr_ps_{ch}")
        for ct in range(2):
            nc.tensor.matmul(nr_ps[:], ones_col[:], x2c[ct][:],
                             start=(ct == 0), stop=(ct == 1))
        nrm_c = work.tile([1, 512], F32, tag="nr", name=f"nrm_c_{ch}")
        nc.scalar.activation(out=nrm_c[:], in_=nr_ps[:], func=AF.Sqrt)
        nc.vector.reciprocal(out=invn_row[0:1, sl], in_=nrm_c[:])
        ncol_ps = psum.tile([128, 4], F32, tag=ptag, name=f"ncol_ps_{ch}")
        for tt in range(4):
            for ct in range(2):
                nc.tensor.matmul(ncol_ps[:, tt:tt + 1],
                                 x2c[ct][:, tt * 128:(tt + 1) * 128].bitcast(F32),
                                 ones_col[:].bitcast(F32),
                                 start=(ct == 0), stop=(ct == 1))
        ncol_sb = work.tile([128, 4], F32, tag="ncs", name=f"ncol_sb_{ch}")
        nc.scalar.activation(out=ncol_sb[:], in_=ncol_ps[:], func=AF.Sqrt)
        nc.vector.reciprocal(out=invn_col[:, ch * 4:(ch + 1) * 4], in_=ncol_sb[:])
        ib_ps = psum.tile([128, 512], F32, tag=ptag, name=f"ib_ps_{ch}")
        nc.tensor.matmul(ib_ps[:], ones_row[:], invn_row[0:1, sl])
        nc.vector.tensor_copy(inv_bcast[:, sl], ib_ps[:])

    def proj_bundle(ch, ptag):
        """q/k/V projections + staging for one 512-col chunk (PE/DVE only)."""
        def emit():
            sl = bass.ts(ch, 512)
            for mt in range(2):
                qk_ps = psum.tile([128, 512], F32, tag=ptag, name=f"qk_ps_{ch}_{mt}")
                for ct in range(2):
                    nc.tensor.matmul(qk_ps[:], wa_sb[:, ct, mt * 128:(mt + 1) * 128],
                                     x_sb[:, ct, sl], start=(ct == 0), stop=(ct == 1))
                nc.vector.tensor_mul(qk_slab[:, mt, sl], qk_ps[:], inv_bcast[:, sl])
            for tt in range(4):
                t = ch * 4 + tt
                v_ps = psum.tile([128, 256], F32, tag=ptag, name=f"v_ps_{t}")
                for ct in range(2):
                    nc.tensor.matmul(v_ps[:], x_sb[:, ct, t * 128:(t + 1) * 128],
                                     wv_sb[:, ct, :], start=(ct == 0), stop=(ct == 1))
                for h in range(2):
                    nc.vector.tensor_scalar_mul(out=v_slab[:, t, h, 0:64],
                                                in0=v_ps[:, h * 64:(h + 1) * 64],
                                                scalar1=invn_col[:, t:t + 1])
        return emit

    def finalize_stage(ic, h, o_ps, o_slab):
        def emit():
            recd = work.tile([1, ICH], F32R, tag="rd", name=f"rd_{ic}_{h}")
            nc.vector.reciprocal(out=recd[:], in_=o_ps[64:65, :])
            bc_ps = psum.tile([64, ICH], F32, tag="s", name=f"bc_ps_{ic}_{h}")
            for hf in range(2):
                nc.tensor.matmul(bc_ps[:, hf * 512:(hf + 1) * 512],
                                 ones_row[0:1, 0:64],
                                 recd[0:1, hf * 512:(hf + 1) * 512])
            bc_sb = work.tile([64, ICH], F32, tag="bc", name=f"bc_sb_{ic}_{h}")
            nc.vector.tensor_copy(bc_sb[:], bc_ps[:])
            nc.vector.tensor_mul(o_slab[:, h, :], o_ps[0:64, :], bc_sb[:])
        return emit

    def outproj_stage(ic, o_slab):
        def emit():
            for mt in range(2):
                for cc in range(2):
                    y_ps = psum.tile([128, 512], F32, tag="o",
                                     name=f"y_ps_{ic}_{mt}_{cc}")
                    for h in range(2):
                        nc.tensor.matmul(y_ps[:],
                                         wo_sb[:, h, mt * 128:(mt + 1) * 128],
                                         o_slab[:, h, cc * 512:(cc + 1) * 512],
                                         start=(h == 0), stop=(h == 1))
                    y_ev = pwork.tile([128, 512], F32, tag="yev",
                                      name=f"y_ev_{ic}_{mt}_{cc}")
                    nc.vector.tensor_copy(y_ev[:], y_ps[:])
                    nc.sync.dma_start(
                        out=y[mt * 128:(mt + 1) * 128,
                              ic * ICH + cc * 512:ic * ICH + (cc + 1) * 512],
                        in_=y_ev[:])
        return emit

    nic = int(os.environ.get("K_NIC", NIC))
    stages = [(ic, h) for ic in range(nic) for h in range(2)]

    # prefix: all norm bundles (the only ACT-Sqrt users) + first two proj bundles
    for ch in range(8):
        norm_bundle(ch, "s" if ch % 2 else "o")
    proj_bundle(0, "s")()
    proj_bundle(1, "o")()

    # deferred emissions keyed by (stage_index, j)
    deferred = {}
    for ch in range(2, 8):
        # K chunk ch is first read by scores at j = 4*ch of stage 0
        deferred.setdefault((0, 4 * ch - 6), []).append(proj_bundle(ch, "o"))

    o_slab = None
    for si, (ic, h) in enumerate(stages):
        if h == 0:
            o_slab = work.tile([64, 2, ICH], F32R, tag="osl", name=f"osl_{ic}")
        hsl = slice(h * 64, (h + 1) * 64)
        o_ps = psum.tile([65, ICH], F32, tag="o", name=f"o_ps_{ic}_{h}")
        pq = {}
        for j in range(NJ):
            s_ps = psum.tile([128, ICH], F32, tag="s", name=f"s_{ic}_{h}_{j}")
            for hf in range(2):
                nc.tensor.matmul(
                    s_ps[:, hf * 512:(hf + 1) * 512],
                    qk_slab[hsl, 1, j * 128:(j + 1) * 128],
                    qk_slab[hsl, 0, ic * ICH + hf * 512:ic * ICH + (hf + 1) * 512],
                )
            for fn in deferred.pop((si, j), []):
                fn()
            if j >= 3:
                for hf in range(2):
                    nc.tensor.matmul(o_ps[:, hf * 512:(hf + 1) * 512],
                                     v_slab[:, j - 3, h, :],
                                     pq[j - 3][:, hf * 512:(hf + 1) * 512],
                                     start=(j - 3 == 0), stop=False)
            p_t = pwork.tile([128, ICH], F32R, tag="p", bufs=4,
                             name=f"p_{ic}_{h}_{j}")
            nc.scalar.activation(out=p_t[:], in_=s_ps[:], func=AF.Exp)
            pq[j] = p_t
        def tail_avs(h, o_ps, pq):
            def emit():
                for jj in (NJ - 3, NJ - 2, NJ - 1):
                    for hf in range(2):
                        nc.tensor.matmul(o_ps[:, hf * 512:(hf + 1) * 512],
                                         v_slab[:, jj, h, :],
                                         pq[jj][:, hf * 512:(hf + 1) * 512],
                                         start=False, stop=(jj == NJ - 1))
            return emit
        deferred.setdefault((si + 1, 0), []).append(tail_avs(h, o_ps, pq))
        deferred.setdefault((si + 1, 2), []).append(
            finalize_stage(ic, h, o_ps, o_slab))
        if h == 1:
            deferred.setdefault((si + 1, 6), []).append(outproj_stage(ic, o_slab))
    for key in sorted(deferred):
        for fn in deferred[key]:
            fn()


_NC_CACHE = None


def _build():
    global _NC_CACHE
    if _NC_CACHE is not None:
        return _NC_CACHE
    nc = bacc.Bacc("TRN2", target_bir_lowering=False, debug=False, num_devices=NCORES)
    xb = nc.dram_tensor("xb", [C, N], BF16, kind="ExternalInput").ap()
    wa = nc.dram_tensor("wa", [C, 256], BF16, kind="ExternalInput").ap()
    wv = nc.dram_tensor("wv", [C, 256], BF16, kind="ExternalInput").ap()
    wo = nc.dram_tensor("wo", [128, C], F32R, kind="ExternalInput").ap()
    y = nc.dram_tensor("y", [C, N], F32, kind="ExternalOutput").ap()
    with tile.TileContext(nc) as tc, ExitStack() as ctx:
        with nc.allow_low_precision(reason="float32r rounding is within tolerance"):
            build_attention_body(nc, tc, ctx, xb, wa, wv, wo, y)
    nc.compile()
    _NC_CACHE = nc
    return nc


def _host_prep(x, g, w_qkv, w_out):
    """Per-core input maps."""
    x = np.ascontiguousarray(np.asarray(x, np.float32))
    g = np.asarray(g, np.float32)
    w_qkv = np.asarray(w_qkv, np.float32)
    w_out = np.asarray(w_out, np.float32)

    Wg = w_qkv * (g * np.sqrt(np.float32(C)))[None, :]
    Wq = Wg[0:HID] * np.float32(DH ** -0.5)
    Wk = Wg[HID:2 * HID]
    Wv = Wg[2 * HID:3 * HID]

    in_maps = []
    for core in range(NCORES):
        b = core // 4
        h0 = 2 * (core % 4)
        sl = slice(h0 * DH, (h0 + 2) * DH)
        W_A = np.concatenate([Wq[sl], Wk[sl]], 0)            # [256, 256]
        wo_slice = w_out[:, sl]                              # [256, 128]
        wo_dev = np.ascontiguousarray(
            wo_slice.T.reshape(2, DH, C).transpose(1, 0, 2).reshape(128, C))
        in_maps.append({
            "xb": np.ascontiguousarray(x[b].reshape(C, N)).astype(_ml.bfloat16),
            "wa": np.ascontiguousarray(W_A.T).astype(_ml.bfloat16),  # [c, o]
            "wv": np.ascontiguousarray(np.pad(Wv[sl].T, ((0, 0), (0, 128)))).astype(_ml.bfloat16),  # [c, 256] zero-padded
            "wo": wo_dev,                                    # [(d,h), c]
        })
    return in_maps


_RUNNER_CACHE = None


def _make_runner(nc):
    """Build the sharded PJRT callable once; reuse across kernel() calls
    (run_bass_kernel_spmd re-traces jax on every invocation)."""
    import jax
    from jax.sharding import Mesh, PartitionSpec
    from jax.experimental.shard_map import shard_map
    from concourse import bass2jax

    bass2jax.install_neuronx_cc_hook()
    in_names, out_names, out_avals, zero_outs = [], [], [], []
    for alloc in nc.m.functions[0].allocations:
        if not isinstance(alloc, mybir.MemoryLocationSet):
            continue
        name = alloc.memorylocations[0].name
        if alloc.kind == "ExternalInput":
            if nc.partition_id_tensor is None or name != nc.partition_id_tensor.name:
                in_names.append(name)
        elif alloc.kind == "ExternalOutput":
            out_names.append(name)
            shape = tuple(alloc.tensor_shape)
            dtype = mybir.dt.np(alloc.dtype)
            out_avals.append(jax.core.ShapedArray(shape, dtype))
            zero_outs.append(np.zeros(shape, dtype))
    n_params = len(in_names)
    all_in_names = list(in_names) + list(out_names)
    if nc.partition_id_tensor is not None:
        all_in_names.append(nc.partition_id_tensor.name)

    def _body(*args):
        operands = list(args)
        if nc.partition_id_tensor is not None:
            operands.append(bass2jax.partition_id_tensor())
        return tuple(bass2jax._bass_exec_p.bind(
            *operands,
            out_avals=tuple(out_avals),
            in_names=tuple(all_in_names),
            out_names=tuple(out_names),
            lowering_input_output_aliases=(),
            sim_require_finite=True,
            sim_require_nnan=True,
            nc=nc,
        ))

    devices = jax.devices()[:NCORES]
    mesh = Mesh(np.asarray(devices), ("core",))
    n_outs = len(out_avals)
    fn = jax.jit(
        shard_map(_body, mesh=mesh,
                  in_specs=(PartitionSpec("core"),) * (n_params + n_outs),
                  out_specs=(PartitionSpec("core"),) * n_outs,
                  check_rep=False),
        keep_unused=True,
    )
    sharding = jax.sharding.NamedSharding(mesh, PartitionSpec("core"))
    dev_zero = [jax.device_put(
        np.zeros((NCORES * z.shape[0], *z.shape[1:]), z.dtype), sharding)
        for z in zero_outs]

    def run(in_maps):
        concat_in = [np.concatenate([np.asarray(m[name]) for m in in_maps], axis=0)
                     for name in in_names]
        dev_in = [jax.device_put(a, sharding) for a in concat_in]
        outs = fn(*dev_in, *dev_zero)
        y = np.asarray(outs[0]).reshape(NCORES, C, N)
        return y

    return run


def kernel(x, g, w_qkv, w_out, b_out):
    global _RUNNER_CACHE
    nc = _build()
    in_maps = _host_prep(x, g, w_qkv, w_out)
    try:
        if _RUNNER_CACHE is None:
            _RUNNER_CACHE = _make_runner(nc)
        y_cores = _RUNNER_CACHE(in_maps)
    except Exception:
        # fallback: the stock (re-tracing) path
        res = run_bass_kernel_spmd(nc, in_maps, core_ids=list(range(NCORES)))
        y_cores = np.stack([res.results[c]["y"] for c in range(NCORES)])
    y = np.zeros((B, C, N), np.float32)
    for core in range(NCORES):
        y[core // 4] += y_cores[core]
    y += np.asarray(b_out, np.float32)[None, :, None]
    return y.reshape(B, C, H, W, D)
